# revision 14
# baseline (speedup 1.0000x reference)
"""CSABlock Trainium2 kernel, plan D.

Core = 2n + h handles sample n, pixel-half h (2048 px): it computes the
maxpool + phi/gT for its local half, exchanges them with the partner core via
two masked ReduceScatters, and runs the full 2048q x 4096k attention for its
query half.

Structure vs the 194us plan-B baseline:
- Feature is staged px-major/d-innermost and streamed with plain HWDGE loads
  issued FIRST on the sync queue (no dependent DMA ahead of them -> no
  head-of-line blocking; baseline lost ~40us to this). The 9-plane maxpool is
  ONE DVE tensor_reduce(max, axis=X) per tile (~39us DVE, vs 44us tree),
  writing x directly as bf16 so phi/g matmuls run bf16 at full PE rate.
- Attention is emitted key-major interleaved with the block pipeline, with
  the two query halves (A/B) as separate PSUM accumulators + softmax states:
  first exp fires ~15us in, and phase A's normalize/output tail overlaps
  phase B's last chunks.
- softmax z: DVE builds bf16 pair/quad partial sums; quads are folded into a
  f32 zacc by gpsimd accum-DMAs (CCE add) -- the 128-partition reduction and
  1/z broadcast then go matmul -> DRAM-bounce as in the baseline.
- ReduceScatters per 1024-px half are staged the moment their phi/g blocks
  exist (~25us / ~45us) so the ~25us CC protocol hides under local attention.
- Engine balance targets: DVE ~88us (pool+z+tail, pacing), ACT ~79us (exps),
  PE ~70us, GpSimd only issues collectives + z accum-DMAs.
"""

import numpy as np
import ml_dtypes

import concourse.bass as bass
import concourse.mybir as mybir
import concourse.tile as tile
from concourse import bacc

F32 = mybir.dt.float32
F32R = mybir.dt.float32r
BF16 = mybir.dt.bfloat16

C = 256
IC = 128
D = 9
HW = 4096
Q = 2048          # query pixels per core, also local key pixels per core
BLOCKS = [(0, 256), (256, 256), (512, 512), (1024, 512), (1536, 512)]
RS_SPAN = [(0, 1024), (1024, 1024)]   # local px ranges exchanged per RS
NCHUNK = HW // 128                    # 32 key chunks over the full image
EXP_BIAS = -30.0
EPS = 1e-5
GROUPS = [[0, 1], [2, 3], [4, 5], [6, 7]]

AF = mybir.ActivationFunctionType


def build(nc):
    # feature staged px-major, d innermost: one tensor_reduce(max) per tile.
    feat_d = nc.dram_tensor("feat", [2, 128, Q, D], F32, kind="ExternalInput")
    center_d = nc.dram_tensor("center", [2, 128, Q], F32R, kind="ExternalInput")
    wthT_d = nc.dram_tensor("wthT", [2, 128, 128], F32R, kind="ExternalInput")
    wphT_d = nc.dram_tensor("wphT", [2, 128, 128], BF16, kind="ExternalInput")
    wgT_d = nc.dram_tensor("wgT", [2, 128, 128], BF16, kind="ExternalInput")
    wwT_d = nc.dram_tensor("wwT", [2, 128, 128], F32R, kind="ExternalInput")
    bn_d = nc.dram_tensor("bnpack", [128, 8], F32, kind="ExternalInput")
    mask_d = nc.dram_tensor("mask", [128, 2], F32, kind="ExternalInput")
    out_d = nc.dram_tensor("out", [2, 128, 2, 1024], F32, kind="ExternalOutput")

    # exchange buffers: per RS half r, slot s: [:, 0:1024] = phi, [:, 1024:]
    # = gT (bf16, mask-selected: real data only in the partner's slot, zeros
    # in our own so the RS add yields exactly the partner's half).
    pb2_d = [nc.dram_tensor(f"pb2_{r}", [2, 128, 2048], BF16) for r in range(2)]
    rs_d = [nc.dram_tensor(f"rs{r}", [128, 2048], BF16) for r in range(2)]

    with tile.TileContext(nc) as tc:
        with (
            tc.tile_pool(name="persist", bufs=1) as pp,
            tc.tile_pool(name="tmp", bufs=4) as tp,
            tc.tile_pool(name="fstream", bufs=3) as fp,
            tc.tile_pool(name="xtmp", bufs=2) as xp,
            tc.tile_pool(name="et", bufs=4) as ep,
            tc.tile_pool(name="zt", bufs=2) as zp,
            tc.tile_pool(name="psacc", bufs=1, space="PSUM") as pacc,
            tc.tile_pool(name="pssc", bufs=2, space="PSUM") as psc,
            tc.tile_pool(name="dram", bufs=1, space="DRAM") as dp,
        ):
            # ---- small loads then center then the whole feature stream, all
            # on the sync HWDGE queue with no dependent DMA in between ----
            center_sb = pp.tile([128, 2, Q], F32R)
            wthT = pp.tile([128, 2, 128], F32R)
            wphT = pp.tile([128, 2, 128], BF16)
            wgT = pp.tile([128, 2, 128], BF16)
            wwT = pp.tile([128, 2, 128], F32R)
            bn = pp.tile([128, 8], F32)
            maskv = pp.tile([128, 2], F32)
            nc.sync.dma_start(out=bn[:], in_=bn_d[:])
            nc.sync.dma_start(out=maskv[:], in_=mask_d[:])
            for cc in range(2):
                nc.sync.dma_start(out=wthT[:, cc, :], in_=wthT_d[cc])
                nc.sync.dma_start(out=wphT[:, cc, :], in_=wphT_d[cc])
                nc.sync.dma_start(out=wgT[:, cc, :], in_=wgT_d[cc])
                nc.sync.dma_start(out=wwT[:, cc, :], in_=wwT_d[cc])
            for cc in range(2):
                nc.scalar.dma_start(out=center_sb[:, cc, :], in_=center_d[cc])

            xb = []
            for i, (_, w) in enumerate(BLOCKS):
                xblk = pp.tile([128, 2, w], BF16, tag=f"xb{i}")
                xb.append(xblk)
            ft_tiles = []
            for i, (off, w) in enumerate(BLOCKS):
                for cc in range(2):
                    ft = fp.tile([128, 512, D], F32, tag="ft")
                    nc.sync.dma_start(
                        out=ft[:, 0:w, :], in_=feat_d[cc][:, off : off + w, :]
                    )
                    ft_tiles.append(ft)

            def pool_block(i):
                off, w = BLOCKS[i]
                for cc in range(2):
                    ft = ft_tiles[2 * i + cc]
                    nc.vector.tensor_reduce(
                        out=xb[i][:, cc, :], in_=ft[:, 0:w, :],
                        op=mybir.AluOpType.max, axis=mybir.AxisListType.X,
                    )

            ones32 = pp.tile([128, 1], F32)
            nc.vector.memset(ones32, 1.0)
            eps_t = pp.tile([128, 1], F32)
            nc.vector.memset(eps_t, EPS)
            expb = pp.tile([128, 1], F32)
            nc.vector.memset(expb, EXP_BIAS)
            zacc = []
            for hf in range(2):
                zacc_t = pp.tile([128, 1024], F32, tag=f"zacc{hf}")
                nc.vector.memset(zacc_t, 0.0)
                zacc.append(zacc_t)

            # ---- BN folding (tiny, on ACT while DMA streams) ----
            sc_th = pp.tile([128, 1], F32)
            bi_th = pp.tile([128, 1], F32)
            sc_ph = pp.tile([128, 1], F32)
            bi_ph = pp.tile([128, 1], F32)
            lnvs = []
            for i, o in enumerate((0, 4)):
                lnv = tp.tile([128, 1], F32, tag=f"bnln{i}")
                nc.scalar.activation(lnv, bn[:, o + 3 : o + 4], AF.Ln, bias=eps_t[:])
                lnvs.append(lnv)
            for i, (o, sc_t, bi_t) in enumerate(((0, sc_th, bi_th), (4, sc_ph, bi_ph))):
                rsq = tp.tile([128, 1], F32, tag=f"bnrs{i}")
                nc.scalar.activation(rsq, lnvs[i], AF.Exp, scale=-0.5)
                nc.vector.tensor_mul(sc_t, bn[:, o : o + 1], rsq)
                ms = tp.tile([128, 1], F32, tag=f"bnms{i}")
                nc.vector.tensor_mul(ms, bn[:, o + 2 : o + 3], sc_t)
                nc.vector.tensor_sub(bi_t, bn[:, o + 1 : o + 2], ms)

            # ---- theta (both query halves up front) ----
            theta = pp.tile([128, Q], BF16)
            for hf in range(2):
                ps_t = psc.tile([128, 1024], F32, tag="sc")
                for cc in range(2):
                    for qc in range(2):
                        o = hf * 1024 + qc * 512
                        nc.tensor.matmul(
                            ps_t[:, qc * 512 : (qc + 1) * 512],
                            lhsT=wthT[:, cc, :],
                            rhs=center_sb[:, cc, o : o + 512],
                            start=(cc == 0),
                            stop=(cc == 1),
                        )
                nc.scalar.activation(
                    theta[:, hf * 1024 : (hf + 1) * 1024], ps_t, AF.Relu,
                    bias=bi_th[:], scale=sc_th[:],
                )

            # ---- persistent attention state ----
            pk_phi = pp.tile([128, Q], BF16)
            pk_g = pp.tile([128, Q], BF16)
            phi_rem = pp.tile([128, 2, 1024], BF16)
            gT_rem = pp.tile([128, 2, 1024], BF16)
            out_acc = []
            for hf in range(2):
                oacc_t = pacc.tile([128, 1024], F32, tag=f"acc{hf}")
                out_acc.append(oacc_t)

            # ---- local phi/gT per block ----
            def make_block(i):
                off, w = BLOCKS[i]
                ps_ph = psc.tile([128, 512], F32, tag="sc")
                for cc in range(2):
                    nc.tensor.matmul(
                        ps_ph[:, 0:w],
                        lhsT=wphT[:, cc, :],
                        rhs=xb[i][:, cc, :],
                        start=(cc == 0),
                        stop=(cc == 1),
                    )
                nc.scalar.activation(
                    pk_phi[:, off : off + w], ps_ph[:, 0:w], AF.Relu,
                    bias=bi_ph[:], scale=sc_ph[:],
                )
                ps_g = psc.tile([128, 512], F32, tag="sc")
                for j in range(w // 128):
                    for cc in range(2):
                        nc.tensor.matmul(
                            ps_g[:, j * 128 : (j + 1) * 128],
                            lhsT=xb[i][:, cc, j * 128 : (j + 1) * 128],
                            rhs=wgT[:, cc, :],
                            start=(cc == 0),
                            stop=(cc == 1),
                        )
                nc.vector.tensor_copy(pk_g[:, off : off + w], ps_g[:, 0:w])

            # ---- stage + kick one ReduceScatter (mask-selected slots) ----
            def kick_rs(r):
                off, w = RS_SPAN[r]
                for s in range(2):
                    pkm_phi = xp.tile([128, 1024], BF16, tag="pkmp")
                    pkm_g = xp.tile([128, 1024], BF16, tag="pkmg")
                    mk = maskv[:, s : s + 1]
                    nc.scalar.mul(pkm_phi, pk_phi[:, off : off + w], mk)
                    nc.scalar.mul(pkm_g, pk_g[:, off : off + w], mk)
                    nc.gpsimd.dma_start(out=pb2_d[r][s][:, 0:1024], in_=pkm_phi[:])
                    nc.gpsimd.dma_start(out=pb2_d[r][s][:, 1024:2048], in_=pkm_g[:])
                nc.gpsimd.collective_compute(
                    "ReduceScatter", mybir.AluOpType.add, replica_groups=GROUPS,
                    ins=[pb2_d[r].ap().opt()], outs=[rs_d[r].ap().opt()],
                )

            # ---- attention, key-major, phases = query halves ----
            zstate = [{"idx": 0, "pend": [], "pairs": []} for _ in range(2)]

            def attn_chunk(phis, gts, j):
                ph = phis[:, j * 128 : (j + 1) * 128]
                gt = gts[:, j * 128 : (j + 1) * 128]
                ets = []
                for hf in range(2):
                    s_ps = psc.tile([128, 1024], F32, tag="sc")
                    for qc in range(2):
                        o = hf * 1024 + qc * 512
                        nc.tensor.matmul(
                            s_ps[:, qc * 512 : (qc + 1) * 512],
                            lhsT=ph,
                            rhs=theta[:, o : o + 512],
                            start=True,
                            stop=True,
                        )
                    et = ep.tile([128, 1024], BF16, tag=f"et{hf}")
                    nc.scalar.activation(et, s_ps, AF.Exp, bias=expb[:])
                    ets.append(et)
                for hf in range(2):
                    st = zstate[hf]
                    k = st["idx"]
                    st["idx"] = k + 1
                    for qc in range(2):
                        nc.tensor.matmul(
                            out_acc[hf][:, qc * 512 : (qc + 1) * 512],
                            lhsT=gt,
                            rhs=ets[hf][:, qc * 512 : (qc + 1) * 512],
                            start=(k == 0),
                            stop=(k == NCHUNK - 1),
                        )
                    st["pend"].append(ets[hf])
                    if len(st["pend"]) == 2:
                        e0, e1 = st["pend"]
                        st["pend"] = []
                        pr = zp.tile([128, 1024], BF16, tag=f"pair{hf}")
                        nc.vector.tensor_add(pr, e0, e1)
                        st["pairs"].append(pr)
                    if len(st["pairs"]) == 2:
                        p0, p1 = st["pairs"]
                        st["pairs"] = []
                        qd = zp.tile([128, 1024], BF16, tag=f"quad{hf}")
                        nc.vector.tensor_add(qd, p0, p1)
                        # fold the quad into the f32 z accumulator via a
                        # gpsimd accum-DMA (CCE add, casts bf16->f32)
                        nc.gpsimd.dma_start(
                            out=zacc[hf][:], in_=qd[:],
                            accum_op=mybir.AluOpType.add,
                        )

            # ---- pipeline: blocks -> phi/g -> chunks, RS kicks asap ----
            pool_block(0); make_block(0)
            for j in range(2):
                attn_chunk(pk_phi, pk_g, j)
            pool_block(1); make_block(1)
            for j in range(2, 4):
                attn_chunk(pk_phi, pk_g, j)
            pool_block(2); make_block(2)
            kick_rs(0)
            for j in range(4, 8):
                attn_chunk(pk_phi, pk_g, j)
            pool_block(3); make_block(3)
            for j in range(8, 12):
                attn_chunk(pk_phi, pk_g, j)
            pool_block(4); make_block(4)
            kick_rs(1)
            for j in range(12, 16):
                attn_chunk(pk_phi, pk_g, j)
            for r in range(2):
                nc.gpsimd.dma_start(out=phi_rem[:, r, :], in_=rs_d[r][:, 0:1024])
                nc.gpsimd.dma_start(out=gT_rem[:, r, :], in_=rs_d[r][:, 1024:2048])
                for j in range(8):
                    attn_chunk(phi_rem[:, r, :], gT_rem[:, r, :], j)

            # ---- tail: z-normalize + w_w + residual + store, with the two
            # phase tails step-interleaved so their DMA bounces pipeline ----
            zrow_sb = []
            for hf in range(2):
                zrow = psc.tile([1, 1024], F32, tag="sc")
                for qc in range(2):
                    nc.tensor.matmul(
                        zrow[:, qc * 512 : (qc + 1) * 512],
                        lhsT=ones32[:, 0:1],
                        rhs=zacc[hf][:, qc * 512 : (qc + 1) * 512],
                        start=True,
                        stop=True,
                    )
                zr = pp.tile([1, 1024], F32, tag=f"zrow{hf}")
                nc.vector.tensor_copy(zr, zrow)
                zrow_sb.append(zr)
            zbs = []
            for hf in range(2):
                zb = dp.tile([1, 1024], F32, tag=f"zb{hf}")
                nc.sync.dma_start(out=zb[:], in_=zrow_sb[hf][:])
                zbs.append(zb)
            zbcs = []
            for hf in range(2):
                zb = zbs[hf]
                zb_b = bass.AP(
                    tensor=zb.tensor, offset=zb.offset,
                    ap=[[0, 128]] + [list(p) for p in zb.ap[1:]],
                )
                zbc = xp.tile([128, 1024], F32, tag="invz")
                nc.sync.dma_start(out=zbc[:], in_=zb_b)
                zbcs.append(zbc)
            wsbs = []
            for hf in range(2):
                invz = pp.tile([128, 1024], F32, tag=f"iz{hf}")
                nc.vector.reciprocal(invz, zbcs[hf])
                wsb = pp.tile([128, 1024], F32R, tag=f"wsb{hf}")
                nc.vector.tensor_mul(wsb, out_acc[hf], invz[:])
                wsbs.append(wsb)
            for hf in range(2):
                for oc in range(2):
                    ps_o = psc.tile([128, 1024], F32, tag="sc")
                    for qc in range(2):
                        nc.tensor.matmul(
                            ps_o[:, qc * 512 : (qc + 1) * 512],
                            lhsT=wwT[:, oc, :],
                            rhs=wsbs[hf][:, qc * 512 : (qc + 1) * 512],
                            start=True,
                            stop=True,
                        )
                    onrm = xp.tile([128, 1024], F32, tag="onrm")
                    nc.vector.tensor_add(
                        onrm, ps_o, center_sb[:, oc, hf * 1024 : (hf + 1) * 1024]
                    )
                    nc.sync.dma_start(out=out_d[oc, :, hf, :], in_=onrm[:])


def shard_inputs(inputs):
    f32 = np.float32
    feature = np.asarray(inputs["feature"], dtype=f32)
    w_theta = np.asarray(inputs["w_theta"], dtype=f32)
    w_phi = np.asarray(inputs["w_phi"], dtype=f32)
    w_g = np.asarray(inputs["w_g"], dtype=f32)
    w_w = np.asarray(inputs["w_w"], dtype=f32)
    wthT = np.ascontiguousarray(w_theta.T.reshape(2, 128, 128))
    wphT = np.ascontiguousarray(w_phi.T.reshape(2, 128, 128)).astype(ml_dtypes.bfloat16)
    wgT = np.ascontiguousarray(w_g.T.reshape(2, 128, 128)).astype(ml_dtypes.bfloat16)
    wwT = np.ascontiguousarray(w_w.T.reshape(128, 2, 128).transpose(1, 0, 2))
    bnpack = np.ascontiguousarray(np.stack(
        [
            np.asarray(inputs["bn_theta_gamma"], f32),
            np.asarray(inputs["bn_theta_beta"], f32),
            np.asarray(inputs["bn_theta_mean"], f32),
            np.asarray(inputs["bn_theta_var"], f32),
            np.asarray(inputs["bn_phi_gamma"], f32),
            np.asarray(inputs["bn_phi_beta"], f32),
            np.asarray(inputs["bn_phi_mean"], f32),
            np.asarray(inputs["bn_phi_var"], f32),
        ],
        axis=1,
    ))

    in_maps = []
    for core in range(8):
        n, h = core // 2, core % 2
        # [2, 128, 9, Q] local half -> px-major d-innermost [2, 128, Q, 9]
        fh = feature[n].reshape(2, 128, D, HW)[:, :, :, h * Q : (h + 1) * Q]
        feat = np.ascontiguousarray(fh.transpose(0, 1, 3, 2))
        center = np.ascontiguousarray(
            feature[n][:, D // 2 + 1].reshape(256, HW)[:, h * Q : (h + 1) * Q]
            .reshape(2, 128, Q)
        )
        mask = np.zeros((128, 2), dtype=np.float32)
        mask[:, 1 - h] = 1.0
        in_maps.append(
            dict(feat=feat, center=center, wthT=wthT, wphT=wphT, wgT=wgT,
                 wwT=wwT, bnpack=bnpack, mask=mask)
        )
    return in_maps


def unshard_output(results, N=4):
    out = np.empty((N, 256, 64, 64), dtype=np.float32)
    flat = out.reshape(N, 256, HW)
    for core in range(8):
        n, qh = core // 2, core % 2
        o = results[core]["out"].reshape(256, Q)
        flat[n][:, qh * Q : (qh + 1) * Q] = o
    return out


def make_nc():
    nc = bacc.Bacc("TRN2", target_bir_lowering=False, debug=False, num_devices=8)
    build(nc)
    nc.compile()
    return nc


# ---------------------------------------------------------------------------
# Public entrypoint: full (unsharded) inputs -> full output, running the Bass
# kernel SPMD across the 8 NeuronCores.
# ---------------------------------------------------------------------------
from concourse.bass_utils import run_bass_kernel_spmd

_NC_CACHE = []


def _get_nc():
    if not _NC_CACHE:
        _NC_CACHE.append(make_nc())
    return _NC_CACHE[0]


def kernel(**inputs):
    nc = _get_nc()
    in_maps = shard_inputs(inputs)
    res = run_bass_kernel_spmd(nc, in_maps, list(range(8)))
    return unshard_output(res.results)


# revision 15
# speedup vs baseline: 1.2090x; 1.2090x over previous
"""CSABlock Trainium2 kernel, plan D.

Core = 2n + h handles sample n, pixel-half h (2048 px): it computes the
maxpool + phi/gT for its local half, exchanges them with the partner core via
two masked ReduceScatters, and runs the full 2048q x 4096k attention for its
query half.

Structure vs the 194us plan-B baseline:
- Feature is staged px-major/d-innermost and streamed with plain HWDGE loads
  issued FIRST on the sync queue (no dependent DMA ahead of them -> no
  head-of-line blocking; baseline lost ~40us to this). The 9-plane maxpool is
  ONE DVE tensor_reduce(max, axis=X) per tile (~39us DVE, vs 44us tree),
  writing x directly as bf16 so phi/g matmuls run bf16 at full PE rate.
- Attention is emitted key-major interleaved with the block pipeline, with
  the two query halves (A/B) as separate PSUM accumulators + softmax states:
  first exp fires ~15us in, and phase A's normalize/output tail overlaps
  phase B's last chunks.
- softmax z: DVE builds bf16 pair/quad partial sums; quads are folded into a
  f32 zacc by gpsimd accum-DMAs (CCE add) -- the 128-partition reduction and
  1/z broadcast then go matmul -> DRAM-bounce as in the baseline.
- ReduceScatters per 1024-px half are staged the moment their phi/g blocks
  exist (~25us / ~45us) so the ~25us CC protocol hides under local attention.
- Engine balance targets: DVE ~88us (pool+z+tail, pacing), ACT ~79us (exps),
  PE ~70us, GpSimd only issues collectives + z accum-DMAs.
"""

import numpy as np
import ml_dtypes

import concourse.bass as bass
import concourse.mybir as mybir
import concourse.tile as tile
from concourse import bacc

F32 = mybir.dt.float32
F32R = mybir.dt.float32r
BF16 = mybir.dt.bfloat16

C = 256
IC = 128
D = 9
HW = 4096
Q = 2048          # query pixels per core, also local key pixels per core
BLOCKS = [(0, 256), (256, 256), (512, 512), (1024, 512), (1536, 512)]
RS_SPAN = [(0, 1024), (1024, 1024)]   # local px ranges exchanged per RS
NCHUNK = HW // 128                    # 32 key chunks over the full image
EXP_BIAS = -30.0
EPS = 1e-5
GROUPS = [[0, 1], [2, 3], [4, 5], [6, 7]]

AF = mybir.ActivationFunctionType


def build(nc):
    # feature staged px-major, d innermost: one tensor_reduce(max) per tile.
    feat_d = nc.dram_tensor("feat", [2, 128, Q, D], BF16, kind="ExternalInput")
    center_d = nc.dram_tensor("center", [2, 128, Q], BF16, kind="ExternalInput")
    wthT_d = nc.dram_tensor("wthT", [2, 128, 128], BF16, kind="ExternalInput")
    wphT_d = nc.dram_tensor("wphT", [2, 128, 128], BF16, kind="ExternalInput")
    wgT_d = nc.dram_tensor("wgT", [2, 128, 128], BF16, kind="ExternalInput")
    wwT_d = nc.dram_tensor("wwT", [2, 128, 128], F32R, kind="ExternalInput")
    bn_d = nc.dram_tensor("bnpack", [128, 8], F32, kind="ExternalInput")
    mask_d = nc.dram_tensor("mask", [128, 2], F32, kind="ExternalInput")
    out_d = nc.dram_tensor("out", [2, 128, 2, 1024], F32, kind="ExternalOutput")

    # exchange buffers: per RS half r, slot s: [:, 0:1024] = phi, [:, 1024:]
    # = gT (bf16, mask-selected: real data only in the partner's slot, zeros
    # in our own so the RS add yields exactly the partner's half).
    pb2_d = [nc.dram_tensor(f"pb2_{r}", [2, 128, 2048], BF16) for r in range(2)]
    rs_d = [nc.dram_tensor(f"rs{r}", [128, 2048], BF16) for r in range(2)]

    with tile.TileContext(nc) as tc:
        with (
            tc.tile_pool(name="persist", bufs=1) as pp,
            tc.tile_pool(name="tmp", bufs=4) as tp,
            tc.tile_pool(name="fstream", bufs=4) as fp,
            tc.tile_pool(name="xtmp", bufs=2) as xp,
            tc.tile_pool(name="et", bufs=4) as ep,
            tc.tile_pool(name="zt", bufs=2) as zp,
            tc.tile_pool(name="psacc", bufs=1, space="PSUM") as pacc,
            tc.tile_pool(name="pssc", bufs=2, space="PSUM") as psc,
            tc.tile_pool(name="dram", bufs=1, space="DRAM") as dp,
        ):
            # ---- small loads then center then the whole feature stream, all
            # on the sync HWDGE queue with no dependent DMA in between ----
            center_sb = pp.tile([128, 2, Q], BF16)
            wthT = pp.tile([128, 2, 128], BF16)
            wphT = pp.tile([128, 2, 128], BF16)
            wgT = pp.tile([128, 2, 128], BF16)
            wwT = pp.tile([128, 2, 128], F32R)
            bn = pp.tile([128, 8], F32)
            maskv = pp.tile([128, 2], F32)
            nc.sync.dma_start(out=bn[:], in_=bn_d[:])
            nc.sync.dma_start(out=maskv[:], in_=mask_d[:])
            for cc in range(2):
                nc.sync.dma_start(out=wthT[:, cc, :], in_=wthT_d[cc])
                nc.sync.dma_start(out=wphT[:, cc, :], in_=wphT_d[cc])
                nc.sync.dma_start(out=wgT[:, cc, :], in_=wgT_d[cc])
                nc.sync.dma_start(out=wwT[:, cc, :], in_=wwT_d[cc])
            for cc in range(2):
                nc.scalar.dma_start(out=center_sb[:, cc, :], in_=center_d[cc])

            xb = []
            for i, (_, w) in enumerate(BLOCKS):
                xblk = pp.tile([128, 2, w], BF16, tag=f"xb{i}")
                xb.append(xblk)
            ft_tiles = []
            for i, (off, w) in enumerate(BLOCKS):
                for cc in range(2):
                    ft = fp.tile([128, 512, D], BF16, tag="ft")
                    nc.sync.dma_start(
                        out=ft[:, 0:w, :], in_=feat_d[cc][:, off : off + w, :]
                    )
                    ft_tiles.append(ft)

            def pool_block(i):
                off, w = BLOCKS[i]
                for cc in range(2):
                    ft = ft_tiles[2 * i + cc]
                    nc.vector.tensor_reduce(
                        out=xb[i][:, cc, :], in_=ft[:, 0:w, :],
                        op=mybir.AluOpType.max, axis=mybir.AxisListType.X,
                    )

            ones32 = pp.tile([128, 1], F32)
            nc.vector.memset(ones32, 1.0)
            eps_t = pp.tile([128, 1], F32)
            nc.vector.memset(eps_t, EPS)
            expb = pp.tile([128, 1], F32)
            nc.vector.memset(expb, EXP_BIAS)
            zacc = []
            for hf in range(2):
                zacc_t = pp.tile([128, 1024], F32, tag=f"zacc{hf}")
                nc.vector.memset(zacc_t, 0.0)
                zacc.append(zacc_t)

            # ---- BN folding (tiny, on ACT while DMA streams) ----
            sc_th = pp.tile([128, 1], F32)
            bi_th = pp.tile([128, 1], F32)
            sc_ph = pp.tile([128, 1], F32)
            bi_ph = pp.tile([128, 1], F32)
            lnvs = []
            for i, o in enumerate((0, 4)):
                lnv = tp.tile([128, 1], F32, tag=f"bnln{i}")
                nc.scalar.activation(lnv, bn[:, o + 3 : o + 4], AF.Ln, bias=eps_t[:])
                lnvs.append(lnv)
            for i, (o, sc_t, bi_t) in enumerate(((0, sc_th, bi_th), (4, sc_ph, bi_ph))):
                rsq = tp.tile([128, 1], F32, tag=f"bnrs{i}")
                nc.scalar.activation(rsq, lnvs[i], AF.Exp, scale=-0.5)
                nc.vector.tensor_mul(sc_t, bn[:, o : o + 1], rsq)
                ms = tp.tile([128, 1], F32, tag=f"bnms{i}")
                nc.vector.tensor_mul(ms, bn[:, o + 2 : o + 3], sc_t)
                nc.vector.tensor_sub(bi_t, bn[:, o + 1 : o + 2], ms)

            # ---- theta (both query halves up front) ----
            theta = pp.tile([128, Q], BF16)
            for hf in range(2):
                ps_t = psc.tile([128, 1024], F32, tag="sc")
                for cc in range(2):
                    for qc in range(2):
                        o = hf * 1024 + qc * 512
                        nc.tensor.matmul(
                            ps_t[:, qc * 512 : (qc + 1) * 512],
                            lhsT=wthT[:, cc, :],
                            rhs=center_sb[:, cc, o : o + 512],
                            start=(cc == 0),
                            stop=(cc == 1),
                        )
                nc.scalar.activation(
                    theta[:, hf * 1024 : (hf + 1) * 1024], ps_t, AF.Relu,
                    bias=bi_th[:], scale=sc_th[:],
                )

            # ---- persistent attention state ----
            pk_phi = pp.tile([128, Q], BF16)
            pk_g = pp.tile([128, Q], BF16)
            phi_rem = pp.tile([128, 2, 1024], BF16)
            gT_rem = pp.tile([128, 2, 1024], BF16)
            out_acc = []
            for hf in range(2):
                oacc_t = pacc.tile([128, 1024], F32, tag=f"acc{hf}")
                out_acc.append(oacc_t)

            # ---- local phi/gT per block ----
            def make_block(i):
                off, w = BLOCKS[i]
                ps_ph = psc.tile([128, 512], F32, tag="sc")
                for cc in range(2):
                    nc.tensor.matmul(
                        ps_ph[:, 0:w],
                        lhsT=wphT[:, cc, :],
                        rhs=xb[i][:, cc, :],
                        start=(cc == 0),
                        stop=(cc == 1),
                    )
                nc.scalar.activation(
                    pk_phi[:, off : off + w], ps_ph[:, 0:w], AF.Relu,
                    bias=bi_ph[:], scale=sc_ph[:],
                )
                ps_g = psc.tile([128, 512], F32, tag="sc")
                for j in range(w // 128):
                    for cc in range(2):
                        nc.tensor.matmul(
                            ps_g[:, j * 128 : (j + 1) * 128],
                            lhsT=xb[i][:, cc, j * 128 : (j + 1) * 128],
                            rhs=wgT[:, cc, :],
                            start=(cc == 0),
                            stop=(cc == 1),
                        )
                nc.vector.tensor_copy(pk_g[:, off : off + w], ps_g[:, 0:w])

            # ---- stage + kick one ReduceScatter (mask-selected slots) ----
            def kick_rs(r):
                off, w = RS_SPAN[r]
                ctx = tc.high_priority()
                ctx.__enter__()
                for s in range(2):
                    pkm_phi = xp.tile([128, 1024], BF16, tag="pkmp")
                    pkm_g = xp.tile([128, 1024], BF16, tag="pkmg")
                    mk = maskv[:, s : s + 1]
                    nc.scalar.mul(pkm_phi, pk_phi[:, off : off + w], mk)
                    nc.scalar.mul(pkm_g, pk_g[:, off : off + w], mk)
                    nc.gpsimd.dma_start(out=pb2_d[r][s][:, 0:1024], in_=pkm_phi[:])
                    nc.gpsimd.dma_start(out=pb2_d[r][s][:, 1024:2048], in_=pkm_g[:])
                nc.gpsimd.collective_compute(
                    "ReduceScatter", mybir.AluOpType.add, replica_groups=GROUPS,
                    ins=[pb2_d[r].ap().opt()], outs=[rs_d[r].ap().opt()],
                )
                ctx.__exit__(None, None, None)

            # ---- attention, key-major, phases = query halves ----
            zstate = [{"idx": 0, "pend": [], "pairs": []} for _ in range(2)]

            def attn_chunk(phis, gts, j):
                ph = phis[:, j * 128 : (j + 1) * 128]
                gt = gts[:, j * 128 : (j + 1) * 128]
                ets = []
                for hf in range(2):
                    s_ps = psc.tile([128, 1024], F32, tag="sc")
                    for qc in range(2):
                        o = hf * 1024 + qc * 512
                        nc.tensor.matmul(
                            s_ps[:, qc * 512 : (qc + 1) * 512],
                            lhsT=ph,
                            rhs=theta[:, o : o + 512],
                            start=True,
                            stop=True,
                        )
                    et = ep.tile([128, 1024], BF16, tag=f"et{hf}")
                    nc.scalar.activation(et, s_ps, AF.Exp, bias=expb[:])
                    ets.append(et)
                for hf in range(2):
                    st = zstate[hf]
                    k = st["idx"]
                    st["idx"] = k + 1
                    for qc in range(2):
                        nc.tensor.matmul(
                            out_acc[hf][:, qc * 512 : (qc + 1) * 512],
                            lhsT=gt,
                            rhs=ets[hf][:, qc * 512 : (qc + 1) * 512],
                            start=(k == 0),
                            stop=(k == NCHUNK - 1),
                        )
                    st["pend"].append(ets[hf])
                    if len(st["pend"]) == 2:
                        e0, e1 = st["pend"]
                        st["pend"] = []
                        pr = zp.tile([128, 1024], BF16, tag=f"pair{hf}")
                        nc.vector.tensor_add(pr, e0, e1)
                        st["pairs"].append(pr)
                    if len(st["pairs"]) == 2:
                        p0, p1 = st["pairs"]
                        st["pairs"] = []
                        qd = zp.tile([128, 1024], BF16, tag=f"quad{hf}")
                        nc.vector.tensor_add(qd, p0, p1)
                        # fold the quad into the f32 z accumulator via a
                        # gpsimd accum-DMA (CCE add, casts bf16->f32)
                        nc.gpsimd.dma_start(
                            out=zacc[hf][:], in_=qd[:],
                            accum_op=mybir.AluOpType.add,
                        )

            # ---- pipeline: blocks -> phi/g -> chunks, RS kicks asap ----
            pool_block(0); make_block(0)
            for j in range(2):
                attn_chunk(pk_phi, pk_g, j)
            pool_block(1); make_block(1)
            for j in range(2, 4):
                attn_chunk(pk_phi, pk_g, j)
            pool_block(2); make_block(2)
            kick_rs(0)
            for j in range(4, 8):
                attn_chunk(pk_phi, pk_g, j)
            pool_block(3); make_block(3)
            for j in range(8, 12):
                attn_chunk(pk_phi, pk_g, j)
            pool_block(4); make_block(4)
            kick_rs(1)
            for j in range(12, 16):
                attn_chunk(pk_phi, pk_g, j)
            for r in range(2):
                with tc.high_priority():
                    nc.gpsimd.dma_start(out=phi_rem[:, r, :], in_=rs_d[r][:, 0:1024])
                    nc.gpsimd.dma_start(out=gT_rem[:, r, :], in_=rs_d[r][:, 1024:2048])
                for j in range(8):
                    attn_chunk(phi_rem[:, r, :], gT_rem[:, r, :], j)

            # ---- tail: z-normalize + w_w + residual + store, with the two
            # phase tails step-interleaved so their DMA bounces pipeline ----
            zrow_sb = []
            for hf in range(2):
                zrow = psc.tile([1, 1024], F32, tag="sc")
                for qc in range(2):
                    nc.tensor.matmul(
                        zrow[:, qc * 512 : (qc + 1) * 512],
                        lhsT=ones32[:, 0:1],
                        rhs=zacc[hf][:, qc * 512 : (qc + 1) * 512],
                        start=True,
                        stop=True,
                    )
                zr = pp.tile([1, 1024], F32, tag=f"zrow{hf}")
                nc.vector.tensor_copy(zr, zrow)
                zrow_sb.append(zr)
            zbs = []
            for hf in range(2):
                zb = dp.tile([1, 1024], F32, tag=f"zb{hf}")
                nc.sync.dma_start(out=zb[:], in_=zrow_sb[hf][:])
                zbs.append(zb)
            zcols = []
            for hf in range(2):
                zc = tp.tile([128, 8], F32, tag=f"zcols{hf}")
                nc.sync.dma_start(
                    out=zc[:], in_=zbs[hf].rearrange("o (p c) -> (o p) c", p=128)
                )
                zcols.append(zc)
            izs = []
            for hf in range(2):
                izcols = tp.tile([128, 8], F32, tag=f"izcols{hf}")
                nc.vector.reciprocal(izcols, zcols[hf])
                izs.append(izcols)
            zbis = []
            for hf in range(2):
                zbi = dp.tile([1, 1024], F32, tag=f"zbi{hf}")
                nc.sync.dma_start(
                    out=zbi.rearrange("o (p c) -> (o p) c", p=128), in_=izs[hf][:]
                )
                zbis.append(zbi)
            wsbs = []
            for hf in range(2):
                zbi = zbis[hf]
                zbi_b = bass.AP(
                    tensor=zbi.tensor, offset=zbi.offset,
                    ap=[[0, 128]] + [list(p) for p in zbi.ap[1:]],
                )
                invz = pp.tile([128, 1024], F32, tag=f"iz{hf}")
                nc.sync.dma_start(out=invz[:], in_=zbi_b)
                wsb = pp.tile([128, 1024], F32R, tag=f"wsb{hf}")
                nc.vector.tensor_mul(wsb, out_acc[hf], invz[:])
                wsbs.append(wsb)
            for hf in range(2):
                for oc in range(2):
                    ps_o = psc.tile([128, 1024], F32, tag="sc")
                    for qc in range(2):
                        nc.tensor.matmul(
                            ps_o[:, qc * 512 : (qc + 1) * 512],
                            lhsT=wwT[:, oc, :],
                            rhs=wsbs[hf][:, qc * 512 : (qc + 1) * 512],
                            start=True,
                            stop=True,
                        )
                    onrm = xp.tile([128, 1024], F32, tag="onrm")
                    nc.vector.tensor_add(
                        onrm, ps_o, center_sb[:, oc, hf * 1024 : (hf + 1) * 1024]
                    )
                    nc.sync.dma_start(out=out_d[oc, :, hf, :], in_=onrm[:])


def shard_inputs(inputs):
    f32 = np.float32
    feature = np.asarray(inputs["feature"], dtype=f32)
    w_theta = np.asarray(inputs["w_theta"], dtype=f32)
    w_phi = np.asarray(inputs["w_phi"], dtype=f32)
    w_g = np.asarray(inputs["w_g"], dtype=f32)
    w_w = np.asarray(inputs["w_w"], dtype=f32)
    wthT = np.ascontiguousarray(w_theta.T.reshape(2, 128, 128)).astype(ml_dtypes.bfloat16)
    wphT = np.ascontiguousarray(w_phi.T.reshape(2, 128, 128)).astype(ml_dtypes.bfloat16)
    wgT = np.ascontiguousarray(w_g.T.reshape(2, 128, 128)).astype(ml_dtypes.bfloat16)
    wwT = np.ascontiguousarray(w_w.T.reshape(128, 2, 128).transpose(1, 0, 2))
    bnpack = np.ascontiguousarray(np.stack(
        [
            np.asarray(inputs["bn_theta_gamma"], f32),
            np.asarray(inputs["bn_theta_beta"], f32),
            np.asarray(inputs["bn_theta_mean"], f32),
            np.asarray(inputs["bn_theta_var"], f32),
            np.asarray(inputs["bn_phi_gamma"], f32),
            np.asarray(inputs["bn_phi_beta"], f32),
            np.asarray(inputs["bn_phi_mean"], f32),
            np.asarray(inputs["bn_phi_var"], f32),
        ],
        axis=1,
    ))

    in_maps = []
    for core in range(8):
        n, h = core // 2, core % 2
        # [2, 128, 9, Q] local half -> px-major d-innermost [2, 128, Q, 9]
        fh = feature[n].reshape(2, 128, D, HW)[:, :, :, h * Q : (h + 1) * Q]
        feat = np.ascontiguousarray(fh.transpose(0, 1, 3, 2)).astype(ml_dtypes.bfloat16)
        center = np.ascontiguousarray(
            feature[n][:, D // 2 + 1].reshape(256, HW)[:, h * Q : (h + 1) * Q]
            .reshape(2, 128, Q)
        ).astype(ml_dtypes.bfloat16)
        mask = np.zeros((128, 2), dtype=np.float32)
        mask[:, 1 - h] = 1.0
        in_maps.append(
            dict(feat=feat, center=center, wthT=wthT, wphT=wphT, wgT=wgT,
                 wwT=wwT, bnpack=bnpack, mask=mask)
        )
    return in_maps


def unshard_output(results, N=4):
    out = np.empty((N, 256, 64, 64), dtype=np.float32)
    flat = out.reshape(N, 256, HW)
    for core in range(8):
        n, qh = core // 2, core % 2
        o = results[core]["out"].reshape(256, Q)
        flat[n][:, qh * Q : (qh + 1) * Q] = o
    return out


def make_nc():
    nc = bacc.Bacc("TRN2", target_bir_lowering=False, debug=False, num_devices=8)
    build(nc)
    nc.compile()
    return nc


# ---------------------------------------------------------------------------
# Public entrypoint: full (unsharded) inputs -> full output, running the Bass
# kernel SPMD across the 8 NeuronCores.
# ---------------------------------------------------------------------------
from concourse.bass_utils import run_bass_kernel_spmd

_NC_CACHE = []


def _get_nc():
    if not _NC_CACHE:
        _NC_CACHE.append(make_nc())
    return _NC_CACHE[0]


def kernel(**inputs):
    nc = _get_nc()
    in_maps = shard_inputs(inputs)
    res = run_bass_kernel_spmd(nc, in_maps, list(range(8)))
    return unshard_output(res.results)
